# revision 43
# baseline (speedup 1.0000x reference)
"""ArcDecoder distributed Bass kernel for 8 TRN2 NeuronCores.

Problem: for each arc e with endpoints (s, d):
    h   = concat(z[s], z[d])                # [256]
    h1  = relu(W1 @ h + b1)                 # [128]
    out = W2 @ h1 + b2                      # scalar

Math transform: W1 @ concat(z_s, z_d) = W1a @ z_s + W1b @ z_d, so per-node
tables are precomputed once (100k nodes instead of 1M arcs):
    A~[n] = (z[n] @ W1a.T) * |W2|,  B~[n] = (z[n] @ W1b.T) * |W2| + |W2|*b1
stored interleaved in bf16 as T[n] = [A~[n], B~[n]] (512B rows).  Then
    out[e] = sum_j sign(W2_j) * relu(A~[s,j] + B~[d,j]) + b2
i.e. per arc: two 256B gathers + an add + one fused max0/mul DVE op + a
segmented reduce.  No per-arc matmul.

Gather: `dma_gather` (the Q7 SWDGE gather) takes int16 indices, so nodes are
split into 4 ranges of 25024 rows and each core's arcs are host-bucketed into
16 (src_range, dst_range) groups; each group's gathers use the range base as
the table offset so all indices fit int16.  Groups are padded to a static
capacity with index-0 dummies; the padding is discarded on the host.

Sharding: arcs split evenly across the 8 cores; z/weights replicated.
No collectives.
"""

import numpy as np

# ---------------- problem constants (hardcoded, per the task spec) ----------
N_NODES = 100000
HIDDEN = 128
N_ARCS = 1000000
N_CORES = 8

P = 128  # SBUF partitions

# ---------------- tiling configuration --------------------------------------
NRANGE = 4
# range size tile-aligned so each range's table is whole 128-node tiles
RSIZE = ((N_NODES + NRANGE * P - 1) // (NRANGE * P)) * P  # 25088
NODE_PAD = NRANGE * RSIZE  # 100352
NGRP = NRANGE * NRANGE  # 16

E_PER_CORE = N_ARCS // N_CORES  # 125000
# group capacity: mean count is E_PER_CORE/16 = 7812.5, sigma ~86 for uniform
# random arcs; 64 tiles = 8192 is ~4.4 sigma above the mean (and makes the
# result columns align to whole 128-col transpose blocks: 2 groups/block).
CTILES = 64
CAP = CTILES * P  # 8192
E_OUT = NGRP * CAP  # 133120 device outputs per core

CHUNK_TILES = 16  # node tiles per z-chunk DMA in the precompute phase
WB_TILES = 16  # node tiles per T-writeback DMA (1 MB)

# knobs
ADD_VIA_CCE = False  # A += B via SWDGE SBUF->SBUF accumulate DMA (else DVE)
GATHER_QUEUES = 4  # SWDGE queues to spread dma_gathers over (1..4)


def _build_graph(node_pad, rsize, cap, chunk_tiles, b1_nonzero=False,
                 add_via_cce=ADD_VIA_CCE, gather_queues=GATHER_QUEUES):
    """Build the SPMD single-core graph (all 8 cores run this same graph)."""
    import concourse.bass as bass
    from concourse import bacc, mybir, tile
    from concourse.masks import make_identity

    BF16 = mybir.dt.bfloat16
    F32 = mybir.dt.float32
    I16 = mybir.dt.int16
    H = HIDDEN
    NT = node_pad // P
    nrange = node_pad // rsize
    ngrp = nrange * nrange
    ctiles = cap // P
    S = cap // 16  # idx free-dim per group (16-partition wrap)
    e_out = ngrp * cap
    rcols = ngrp * ctiles  # result columns (= e_out / 128)
    rcols_pad = ((rcols + P - 1) // P) * P

    nc = bacc.Bacc(None, target_bir_lowering=False,
                   num_swdge_queues=gather_queues)
    with tile.TileContext(nc) as tc:
        with tc.tile_pool(name="dram", bufs=1, space="DRAM") as dram:
            z_T = dram.tile([P, node_pad], BF16, kind="ExternalInput",
                            name="z_T", uniquify=False)
            wcat = dram.tile([P, 2 * H], BF16, kind="ExternalInput",
                             name="wcat", uniquify=False)
            beta = dram.tile([P, 2 * H], F32, kind="ExternalInput",
                             name="beta", uniquify=False)
            sgn = dram.tile([P, H], BF16, kind="ExternalInput",
                            name="sgn", uniquify=False)
            b2r = dram.tile([P, 1], F32, kind="ExternalInput",
                            name="b2r", uniquify=False)
            isrc = dram.tile([P, ngrp * S], I16, kind="ExternalInput",
                             name="isrc", uniquify=False)
            idst = dram.tile([P, ngrp * S], I16, kind="ExternalInput",
                             name="idst", uniquify=False)
            outv = dram.tile([e_out], F32, kind="ExternalOutput",
                             name="outv", uniquify=False)
            # partition-major table layout: node (local) n lives at
            # [p = n % 128, t = n // 128] so the phase-1 writeback is
            # contiguous 8KB per partition (128 descriptors per chunk
            # instead of 2048 x 512B row descriptors).  Gather row index
            # for node n is (n % 128) * RT + n // 128 (host-computed).
            RT = rsize // P
            Ttabs = [dram.tile([P, RT, 2 * H], BF16, kind="Internal",
                                name=f"Ttab{r}", uniquify=False)
                     for r in range(nrange)]

            with tc.tile_pool(name="consts", bufs=1) as cpool:
                wcat_s = cpool.tile([P, 2 * H], BF16, name="wcat_s")
                nc.sync.dma_start(out=wcat_s[:], in_=wcat[:])
                beta_s = cpool.tile([P, 2 * H], F32, name="beta_s")
                nc.sync.dma_start(out=beta_s[:], in_=beta[:])
                sgn_s = cpool.tile([P, H], BF16, name="sgn_s")
                nc.sync.dma_start(out=sgn_s[:], in_=sgn[:])
                b2_s = cpool.tile([P, 1], F32, name="b2_s")
                nc.sync.dma_start(out=b2_s[:], in_=b2r[:])

                # ---- Phase 1: per-node tables T = [A~ | B~] ----
                # all pools share one scope: phase-2 tiles must NOT alias
                # phase-1 SBUF (aliasing would serialize the phases and has
                # shown nondeterministic HW crashes)
                with tc.tile_pool(name="zc", bufs=2) as zpool, \
                     tc.tile_pool(name="ps", bufs=3, space="PSUM") as pspool, \
                     tc.tile_pool(name="tt", bufs=2) as ttpool, \
                     tc.tile_pool(name="gx", bufs=4) as gxpool, \
                     tc.tile_pool(name="gy", bufs=4) as gypool, \
                     tc.tile_pool(name="gi", bufs=8) as gipool, \
                     tc.tile_pool(name="res", bufs=1) as rpool, \
                     tc.tile_pool(name="trp", bufs=2, space="PSUM") as trppool, \
                     tc.tile_pool(name="trs", bufs=2) as trspool:
                    RT = rsize // P  # tiles per range
                    for r in range(nrange):
                      for c0 in range(0, RT, chunk_tiles):
                        ct = min(chunk_tiles, RT - c0)
                        g0 = r * RT + c0  # global tile index
                        zc = zpool.tile([P, chunk_tiles * P], BF16, tag="zc")
                        nc.sync.dma_start(out=zc[:, :ct * P],
                                          in_=z_T[:, g0 * P:(g0 + ct) * P])
                        tt = ttpool.tile([P, chunk_tiles, 2 * H], BF16,
                                         tag="tt")
                        for t4 in range(0, ct, 4):
                            n4 = min(4, ct - t4)
                            ps = pspool.tile([P, 4, 2 * H], F32, tag="ps")
                            for t in range(t4, t4 + n4):
                                nc.tensor.matmul(ps[:, t - t4, :],
                                                 lhsT=zc[:, t * P:(t + 1) * P],
                                                 rhs=wcat_s[:],
                                                 start=True, stop=True)
                            # batched PSUM f32 -> SBUF bf16 (+beta when
                            # b1 != 0); alternate DVE/ACT per 4-tile block.
                            dst4 = tt[:, t4:t4 + n4, :]
                            src4 = ps[:, :n4, :]
                            if b1_nonzero:
                                beta_b = beta_s[:].rearrange(
                                    "p (x j) -> p x j", x=1).broadcast_to(
                                    [P, n4, 2 * H])
                                nc.vector.scalar_tensor_tensor(
                                    out=dst4, in0=src4, scalar=1.0,
                                    in1=beta_b,
                                    op0=mybir.AluOpType.mult,
                                    op1=mybir.AluOpType.add)
                            elif (t4 // 4) % 4 == 0:
                                nc.vector.tensor_copy(dst4, src4)
                            else:
                                nc.scalar.copy(dst4, src4)
                        for w0 in range(0, ct, WB_TILES):
                            wt = min(WB_TILES, ct - w0)
                            dst = Ttabs[r][:, c0 + w0:c0 + w0 + wt, :]
                            nc.sync.dma_start(out=dst,
                                              in_=tt[:, w0:w0 + wt, :])

                    # ---- Phase 2: gather + score arcs, 16 (a,b) groups ----
                    resall = rpool.tile([P, rcols_pad], F32, name="resall")
                    nc.vector.memset(resall[:], 0.0)
                    resb = rpool.tile([P, rcols_pad], F32, name="resb")
                    grp_order = sorted(range(ngrp),
                                       key=lambda g: (max(divmod(g, nrange)),
                                                      g))
                    # idx/result layout is processing-order (qi) major; the
                    # host maps arcs via the same grp_order
                    for qi, g in enumerate(grp_order):
                        ga, gb = divmod(g, nrange)
                        ia = gipool.tile([P, S], I16, tag="ia")
                        nc.sync.dma_start(out=ia[:],
                                          in_=isrc[:, qi * S:(qi + 1) * S])
                        ib = gipool.tile([P, S], I16, tag="ib")
                        nc.sync.dma_start(out=ib[:],
                                          in_=idst[:, qi * S:(qi + 1) * S])
                        gA = gxpool.tile([P, ctiles, H], BF16, tag="gA")
                        gB = gypool.tile([P, ctiles, H], BF16, tag="gB")
                        # A-half rows of range ga / B-half rows of range gb
                        srcA = Ttabs[ga][:].rearrange(
                            "p t j -> (p t) j")[:, 0:H]
                        srcB = Ttabs[gb][:].rearrange(
                            "p t j -> (p t) j")[:, H:2 * H]
                        # split each gather across queues so several Q7
                        # core-pairs generate descriptors concurrently.
                        # Queue-0 generation runs INLINE on the Pool engine
                        # (blocks dispatch), queues 1-3 run async on other
                        # Q7 pairs -- so emit the async ops FIRST and give
                        # queue 0 the last sub of each direction.
                        # queue 0 (inline on the Pool engine) gets smaller
                        # subs: the Pool engine also pays dispatch/sem/drain
                        # overhead (~5us/group), so balance 28 tiles inline
                        # vs 33-34 per async queue
                        subs = [(0, 17), (17, 17), (34, 16), (50, 14)]
                        assert sum(n for _, n in subs) == ctiles
                        plan = [("A", 0, 1), ("A", 1, 2), ("B", 0, 3),
                                ("B", 1, 1), ("A", 2, 2), ("B", 2, 3),
                                ("A", 3, 0), ("B", 3, 0)]
                        for d, si, q in plan:
                            t0, nt = subs[si]
                            n_i = nt * P
                            buf, src, idx = ((gA, srcA, ia) if d == "A"
                                             else (gB, srcB, ib))
                            nc.gpsimd.dma_gather(
                                buf[:, t0:t0 + nt, :], src,
                                idx[:, t0 * 8:(t0 + nt) * 8],
                                n_i, n_i, H, elem_step=2 * H,
                                queue_num=q % gather_queues,
                                single_packet=False)
                        # add + relu*sgn + reduce in two halves so the DVE
                        # chain starts before the whole group is gathered
                        # (boundary = end of sub 1 in both directions)
                        for h0, ht in ((0, 34), (34, ctiles - 34)):
                            ga_h = gA[:, h0:h0 + ht, :]
                            nc.vector.tensor_tensor(
                                out=ga_h, in0=ga_h,
                                in1=gB[:, h0:h0 + ht, :],
                                op=mybir.AluOpType.add)
                            sgn_b = sgn_s[:].rearrange(
                                "p (x j) -> p x j", x=1).broadcast_to(
                                [P, ht, H])
                            nc.vector.scalar_tensor_tensor(
                                out=ga_h, in0=ga_h, scalar=0.0, in1=sgn_b,
                                op0=mybir.AluOpType.max,
                                op1=mybir.AluOpType.mult)
                            nc.vector.tensor_reduce(
                                out=resall[:, qi * ctiles + h0:
                                           qi * ctiles + h0 + ht],
                                in_=ga_h, axis=mybir.AxisListType.X,
                                op=mybir.AluOpType.add)

                        # + b2 per completed 128-col block (= 2 groups) so
                        # only the DMA remains after the last group
                        if qi % 2 == 1:
                            c_lo = (qi // 2) * P
                            nc.vector.tensor_scalar_add(
                                out=resb[:, c_lo:c_lo + P],
                                in0=resall[:, c_lo:c_lo + P],
                                scalar1=b2_s[:, 0:1])

                    # single PARTITION-MAJOR output DMA (no device
                    # transpose -- the host absorbs the layout into its
                    # slot map); emitted last so the sync engine never
                    # stalls on DVE-gated stores mid-stream
                    nc.sync.dma_start(
                        out=outv.rearrange("(p c) -> p c", c=rcols_pad),
                        in_=resb[:])
    nc.compile()
    return nc


def _host_prep(z, pot_arcs, W1, b1, W2, b2, n_cores=N_CORES):
    """Stage inputs: dtype/layout conversion, arc bucketing, sharding.

    Returns (in_maps, slot) where slot[i] is the device output position of
    arc i within its core's output vector.
    """
    import ml_dtypes

    bf16 = ml_dtypes.bfloat16
    H = HIDDEN
    z = np.asarray(z, np.float32)
    W1 = np.asarray(W1, np.float32)
    b1 = np.asarray(b1, np.float32).reshape(-1)
    W2 = np.asarray(W2, np.float32).reshape(-1)
    b2 = np.asarray(b2, np.float32).reshape(-1)
    arcs = np.asarray(pot_arcs)

    absw2 = np.abs(W2)
    sgn = np.sign(W2).astype(np.float32)
    wsa = (W1[:, :H] * absw2[:, None]).T  # [i, j]
    wsb = (W1[:, H:] * absw2[:, None]).T
    wcat = np.ascontiguousarray(
        np.concatenate([wsa, wsb], axis=1)).astype(bf16)  # [128, 256]
    beta = np.broadcast_to(
        np.concatenate([np.zeros(H, np.float32), absw2 * b1])[None, :],
        (P, 2 * H)).copy().astype(np.float32)
    sgn_rep = np.broadcast_to(sgn[None, :], (P, H)).copy().astype(bf16)
    b2r = np.full((P, 1), b2[0], np.float32)

    zT = np.zeros((P, NODE_PAD), bf16)
    zT[:, :z.shape[0]] = np.ascontiguousarray(z.T).astype(bf16)

    e_per = arcs.shape[0] // n_cores
    S = CAP // 16
    in_maps = []
    slot_all = np.empty(arcs.shape[0], np.int64)
    for c in range(n_cores):
        sh = arcs[c * e_per:(c + 1) * e_per]
        src = np.asarray(sh[:, 0], np.int64)
        dst = np.asarray(sh[:, 1], np.int64)
        grp = (src // RSIZE) * NRANGE + (dst // RSIZE)
        order = np.argsort(grp, kind="stable")
        counts = np.bincount(grp, minlength=NGRP)
        if counts.max() > CAP:
            raise RuntimeError(f"group overflow: {counts.max()} > {CAP}")
        starts = np.zeros(NGRP, np.int64)
        starts[1:] = np.cumsum(counts)[:-1]
        pos_sorted = np.arange(e_per) - starts[grp[order]]
        # device processes groups in grp_order; idx layout is qi-major by
        # arc slot; outv is PARTITION-major: value for (group qi, pos) at
        # flat index (pos % 128) * (NGRP*CTILES) + qi*CTILES + pos // 128
        grp_order = sorted(range(NGRP),
                           key=lambda g: (max(divmod(g, NRANGE)), g))
        qpos = np.zeros(NGRP, np.int64)
        for qi, g in enumerate(grp_order):
            qpos[g] = qi
        qarr = qpos[grp[order]]
        slot_sorted = qarr * CAP + pos_sorted       # idx-array slot
        oslot_sorted = ((pos_sorted % P) * (NGRP * CTILES)
                        + qarr * CTILES + pos_sorted // P)
        slot = np.empty(e_per, np.int64)
        slot[order] = oslot_sorted
        slot_all[c * e_per:(c + 1) * e_per] = slot

        la = np.zeros(NGRP * CAP, np.int16)  # padding -> local idx 0 (valid)
        lb = np.zeros(NGRP * CAP, np.int16)
        RT = RSIZE // P
        loca = (src - (src // RSIZE) * RSIZE)[order]
        locb = (dst - (dst // RSIZE) * RSIZE)[order]
        # permuted (partition-major) table row index
        la[slot_sorted] = ((loca % P) * RT + loca // P).astype(np.int16)
        lb[slot_sorted] = ((locb % P) * RT + locb // P).astype(np.int16)
        # wrapped idx layout: position i -> (partition i%16, free i//16),
        # per group; replicated across the 8 Q7 core pairs (128 partitions)
        wa = np.ascontiguousarray(
            la.reshape(NGRP, S, 16).transpose(2, 0, 1).reshape(16, NGRP * S))
        wb = np.ascontiguousarray(
            lb.reshape(NGRP, S, 16).transpose(2, 0, 1).reshape(16, NGRP * S))
        in_maps.append(dict(
            z_T=zT, wcat=wcat, beta=beta, sgn=sgn_rep, b2r=b2r,
            isrc=np.tile(wa, (8, 1)), idst=np.tile(wb, (8, 1))))
    return in_maps, slot_all, e_per


_GRAPH_CACHE = {}


def _get_graph(b1_nonzero):
    key = (NODE_PAD, RSIZE, CAP, CHUNK_TILES, b1_nonzero,
           ADD_VIA_CCE, GATHER_QUEUES)
    if key not in _GRAPH_CACHE:
        _GRAPH_CACHE[key] = _build_graph(NODE_PAD, RSIZE, CAP, CHUNK_TILES,
                                         b1_nonzero=b1_nonzero)
    return _GRAPH_CACHE[key]


def kernel(z, pot_arcs, W1, b1, W2, b2):
    from concourse.bass_utils import run_bass_kernel_spmd

    nc = _get_graph(bool(np.any(np.asarray(b1, np.float32))))
    in_maps, slot, e_per = _host_prep(z, pot_arcs, W1, b1, W2, b2)
    res = run_bass_kernel_spmd(nc, in_maps, core_ids=list(range(N_CORES)))
    out = np.empty(N_ARCS, np.float32)
    for c in range(N_CORES):
        dev = np.asarray(res.results[c]["outv"], np.float32)
        out[c * e_per:(c + 1) * e_per] = dev[slot[c * e_per:(c + 1) * e_per]]
    return out

